# revision 46
# baseline (speedup 1.0000x reference)
# Trainium2 Bass kernel for nn_Invo2D, v7: host-transposed xT (no PE
# transposes), row-merged product pieces, DVE/Pool product split, PE/DVE fold
# split, DMA spread across engine queues, fine-grained last-chunk tail.
#
#   t2 = x @ Wc + bc     (Wc = W1@W2, bc = b1@W2+b2 host-folded, bf16)
#   out[pix, co] = sum_j t2[pix, 9*(co//16)+j] * x_tap(f)[pix, ch(f)], f = 9*co+j
#
# Per core (1 image): partition p = hb*8 + wb (16 h-blocks x 8 w-blocks);
# each partition owns a 4x8 pixel block with a 1-pixel 2D halo: free dim =
# (6 h-slots x 10 w-slots x 256 ch).  x zero-padded to [66,66,256] bf16 on
# host.  X flat addr for product f of chunk c at wl: c*2560 + row*1792 +
# wl*256 + f, where row = (f//768); pieces only cut at tap-row boundaries
# (f=768,1536) and group boundaries -> 22 pieces/chunk.
# t2 matmuls use a host-prepared transposed copy xT [2,128,4096] (ch-major),
# so PE does no transposes.  Fold: PE identity-lhsT PSUM-accumulate for co
# [0, TREE_CO0), DVE/Pool bf16 add-tree for the rest; Act evacuates PSUM.
import numpy as np
import ml_dtypes

H, W, C = 64, 64, 256
M144 = 144
NCORES = 8
HS, WS = 6, 10          # h-slots, w-slots per partition (4+2, 8+2)
XF = HS * WS * C        # 15360 bf16 per partition
WLC = 8                 # wl positions per chunk = one hl row
NCHUNK = 4
M9F = WLC * 2304        # 18432
W9F = WLC * M144        # 1152
PW = 66                 # padded row length
XTF = 4096              # pixels per xT half

# fold split: PE folds co [0, TREE_CO0[c]) in 64-blocks, DVE tree the rest.
# Last chunk: PE tail in 16-co blocks with per-block stores and an all-DVE
# tree for the final co range so the tail chain has no cross-engine sems.
TREE_CO0 = (208, 208, 192, 224)
# product piece routing: relative rates (ns per free elem) + per-instr cost
DVE_NS = 0.52
POOL_NS = 0.833
DVE_INSTR_NS = 60.0

_cache = {}


# groups 5 and 10 straddle tap-row boundaries; their rect pieces
# (g, gc0, ngc, j0, nj, row):
_PARTIALS = {
    5: [(5, 0, 5, 0, 9, 0), (5, 5, 1, 0, 3, 0),
        (5, 5, 1, 3, 6, 1), (5, 6, 10, 0, 9, 1)],
    10: [(10, 0, 10, 0, 9, 1), (10, 10, 1, 0, 6, 1),
         (10, 10, 1, 6, 3, 2), (10, 11, 5, 0, 9, 2)],
}
_ROW_OF_FULL = {g: (0 if g < 5 else (1 if g < 10 else 2)) for g in range(16)
                if g not in (5, 10)}

def _build_pieces():
    """Pieces (g, gc0, ngc, j0, nj, row) in f order: full groups stay whole;
    groups 5 and 10 split into rects at tap-row boundaries."""
    pieces = []
    for g in range(16):
        if g in _PARTIALS:
            pieces.extend(_PARTIALS[g])
        else:
            pieces.append((g, 0, 16, 0, 9, _ROW_OF_FULL[g]))
    return pieces


def _route_pieces(pieces, td0=0.0, tp0=0.0):
    """Greedy DVE/Pool routing: assign each piece (f-ascending) to the engine
    with the lower projected finish time.  td0/tp0 seed the per-chunk fold
    burden each engine already carries (DVE: T1+T2 tree, Pool: T3+final).
    DVE pays a per-instruction SBUF-access cost; Pool does not (v1 model)."""
    td, tp = td0, tp0
    routed = []
    for pc in pieces:
        fe = WLC * pc[2] * pc[4]
        if td + fe * DVE_NS + DVE_INSTR_NS <= tp + fe * POOL_NS:
            td += fe * DVE_NS + DVE_INSTR_NS
            routed.append(("dve",) + pc)
        else:
            tp += fe * POOL_NS
            routed.append(("pool",) + pc)
    return routed


def _build_program(n_repeat=1, with_bias=True):
    import concourse.bass as bass
    import concourse.tile as tile
    from concourse import bacc, mybir
    from concourse.masks import make_identity

    f32 = mybir.dt.float32
    bf16 = mybir.dt.bfloat16
    AP = bass.AP

    nc = bacc.Bacc(None, target_bir_lowering=False)
    x_d = nc.dram_tensor("xpad", [PW, PW, C], bf16, kind="ExternalInput")
    xt_d = nc.dram_tensor("xt", [2, 128, XTF], bf16, kind="ExternalInput")
    wc_d = nc.dram_tensor("wc", [2, 128, M144], bf16, kind="ExternalInput")
    bc_d = nc.dram_tensor("bc", [1, M144], bf16, kind="ExternalInput")
    out_d = nc.dram_tensor("out", [128, 32 * C], bf16, kind="ExternalOutput")

    pieces = _build_pieces()
    ROUTED = []
    for c in range(NCHUNK):
        nco_tree = 256 - TREE_CO0[c]
        td0 = (32 + 16) * nco_tree * DVE_NS       # T1 + T2 on DVE
        tp0 = (8 + 8) * nco_tree * POOL_NS        # T3 + final on Pool
        ROUTED.append(_route_pieces(pieces, td0, tp0))

    with tile.TileContext(nc) as tc:
        with (
            tc.tile_pool(name="singles", bufs=1) as singles,
            tc.tile_pool(name="xbufs", bufs=1) as xbufs,
            tc.tile_pool(name="w9p", bufs=1) as w9p,
            tc.tile_pool(name="m9p", bufs=3) as m9p,
            tc.tile_pool(name="treep", bufs=2) as treep,
            tc.tile_pool(name="outp", bufs=2) as outp,
            tc.tile_pool(name="ps2p", bufs=3, space="PSUM") as ps2p,
            tc.tile_pool(name="foldp", bufs=3, space="PSUM") as foldp,
        ):
            ident = singles.tile([128, 128], bf16)
            wc01 = singles.tile([128, 2 * M144], bf16)
            bcsb = singles.tile([1, M144], bf16)
            ones1 = singles.tile([1, 128], bf16)
            XT0 = singles.tile([128, XTF], bf16)
            XT1 = singles.tile([128, XTF], bf16)
            W9c = [w9p.tile([128, W9F], bf16, name=f"w9_{c}", tag=f"w9_{c}")
                   for c in range(NCHUNK)]

            for _rep in range(n_repeat):
                _image_body(nc, bass, mybir, make_identity,
                            xbufs, m9p, treep, outp, ps2p, foldp,
                            ident, wc01, bcsb, ones1, XT0, XT1, W9c,
                            x_d, xt_d, wc_d, bc_d, out_d, ROUTED, with_bias)
    nc.compile()
    return nc


def _image_body(nc, bass, mybir, make_identity,
                xbufs, m9p, treep, outp, ps2p, foldp,
                ident, wc01, bcsb, ones1, XT0, XT1, W9c,
                x_d, xt_d, wc_d, bc_d, out_d, ROUTED, with_bias):
    f32 = mybir.dt.float32
    bf16 = mybir.dt.bfloat16
    AP = bass.AP

    X = xbufs.tile([128, XF], bf16, tag="X")
    XTs = (XT0, XT1)

    def xt_load(eng, h, hl0, nhl):
        eng.dma_start(
            out=AP(XTs[h].tensor, hl0 * 1024, [[XTF, 128], [1, nhl * 1024]]),
            in_=AP(xt_d, h * 128 * XTF + hl0 * 1024,
                   [[XTF, 128], [1, nhl * 1024]]),
        )

    def x_slice(eng, s):
        eng.dma_start(
            out=AP(X.tensor, s * WS * C, [[XF, 128], [1, WS * C]]),
            in_=AP(x_d, s * PW * C,
                   [[4 * PW * C, 16], [8 * C, 8], [1, WS * C]]),
        )

    # -- prologue DMAs spread across queues (SP, Act, Pool) --
    # chunk-c row-r product pieces read only hslot c+r, so for chunk 0 the
    # slice deadlines relax: s0 ~4us, s1 ~5.5us, s2 ~7us.  Act's queue is
    # [wc0, wc1, <auto ATL>, evacs] so the act-table load overlaps t2.
    xt_load(nc.sync, 0, 0, 1)            # SP head: xT half0 row0
    xt_load(nc.scalar, 1, 0, 1)          # Act head: xT half1 row0 (parallel)
    # both wc halves in one DMA on Pool (delivery ~2.4us < first matmul)
    nc.gpsimd.dma_start(
        out=wc01[:],
        in_=AP(wc_d, 0, [[M144, 128], [128 * M144, 2], [1, M144]]),
    )
    if with_bias:
        nc.scalar.dma_start(out=bcsb[:], in_=bc_d[:])
        nc.vector.memset(ones1[:], 1.0)
    make_identity(nc, ident[:])          # Pool, ~0.2us
    x_slice(nc.sync, 0)
    x_slice(nc.sync, 1)
    xt_load(nc.sync, 0, 1, 1)
    xt_load(nc.sync, 1, 1, 1)
    x_slice(nc.sync, 3)
    xt_load(nc.sync, 0, 2, 1)
    xt_load(nc.sync, 1, 2, 1)
    x_slice(nc.sync, 4)
    xt_load(nc.sync, 0, 3, 1)
    xt_load(nc.sync, 1, 3, 1)
    x_slice(nc.sync, 5)

    def t2_grp(r, split_evac=False):
        # 4 pair-tiles; 2 tiles share one PSUM tile, evacuated 288 wide.
        # grp0: evac pairs 0,1 on Act and 2,3 on DVE in parallel (W9c[0]
        # gates all products; DVE idles until then anyway).
        for q in range(4):
            ps2 = ps2p.tile([128, 288], f32, padded_shape=[128, 512])
            for i in range(2):
                t = r * 8 + 2 * q + i
                dst = ps2[:, i * M144:(i + 1) * M144]
                nc.tensor.matmul(dst, lhsT=AP(XT0.tensor, t * 128,
                                              [[XTF, 128], [1, 128]]),
                                 rhs=wc01[:, 0:M144], start=True, stop=False)
                nc.tensor.matmul(dst, lhsT=AP(XT1.tensor, t * 128,
                                              [[XTF, 128], [1, 128]]),
                                 rhs=wc01[:, M144:2 * M144], start=False,
                                 stop=not with_bias)
                if with_bias:
                    nc.tensor.matmul(dst, lhsT=ones1[:], rhs=bcsb[:],
                                     start=False, stop=True)
            w9dst = AP(W9c[r].tensor, 2 * q * M144, [[W9F, 128], [1, 288]])
            if split_evac and q >= 2:
                nc.vector.tensor_copy(out=w9dst, in_=ps2[:])
            else:
                nc.scalar.copy(out=w9dst, in_=ps2[:])

    t2_grp(0, split_evac=True)
    # Act then loads x slice 2 (needed by chunk-0 row-2 products ~8us)
    x_slice(nc.scalar, 2)
    t2_grp(1)

    for c in range(NCHUNK):
        M9 = m9p.tile([128, M9F], bf16, tag="m9")
        # products, f-ascending, routed DVE/Pool.  Chunk 0's first pieces are
        # split into wl halves: the wl 0-3 half only reads W9 pairs 0-1, so
        # products start before the last grp-0 evacuations land.
        for (eng_name, g, gc0, ngc, j0, nj, row) in ROUTED[c]:
            base = c * 2560 + row * 1792
            f0 = 144 * g + 9 * gc0 + j0
            eng = nc.vector if eng_name == "dve" else nc.gpsimd
            halves = ((0, 4), (4, 4)) if (c == 0 and g <= 2) else ((0, WLC),)
            for wl0, nwl in halves:
                eng.tensor_mul(
                    AP(M9.tensor, wl0 * 2304 + f0,
                       [[M9F, 128], [2304, nwl], [9, ngc], [1, nj]]),
                    AP(X.tensor, base + wl0 * 256 + f0,
                       [[XF, 128], [256, nwl], [9, ngc], [1, nj]]),
                    AP(W9c[c].tensor, wl0 * M144 + 9 * g + j0,
                       [[W9F, 128], [M144, nwl], [0, ngc], [1, nj]]),
                )

        outc = outp.tile([128, WLC * 256], bf16, tag="outc")
        tree_co0 = TREE_CO0[c]

        # PE fold blocks for co [0, tree_co0); last chunk uses fine blocks so
        # PE tracks the product stream and the tail chain is short
        pe_blocks = []
        if c == NCHUNK - 1:
            pe_blocks += [(b, 32) for b in range(0, min(192, tree_co0), 32)]
            pe_blocks += [(b, 16) for b in range(192, tree_co0, 16)]
        else:
            for co0 in range(0, tree_co0, 64):
                pe_blocks.append((co0, min(64, tree_co0 - co0)))
        for co0, nco in pe_blocks:
            ps = foldp.tile([128, 512], f32, padded_shape=[128, 512])
            for j in range(9):
                nc.tensor.matmul(
                    ps[:, :WLC * nco],
                    lhsT=ident[:],
                    rhs=AP(M9.tensor, 9 * co0 + j,
                           [[M9F, 128], [2304, WLC], [9, nco]]),
                    start=(j == 0),
                    stop=(j == 8),
                )
            # outc is co-major: outc[p, co*8 + wl]
            nc.scalar.copy(
                out=AP(outc.tensor, co0 * WLC,
                       [[WLC * 256, 128], [1, WLC], [WLC, nco]]),
                in_=AP(ps.tensor, 0, [[512, 128], [1, nco * WLC]]),
            )

        # interleave t2 for the next-next group while folds run
        if c + 2 < NCHUNK:
            t2_grp(c + 2)

        # DVE/Pool add-tree for co [tree_co0, 256); last chunk keeps the
        # whole tree on DVE so the tail chain has no cross-engine sems
        last = c == NCHUNK - 1
        t34eng = nc.vector if last else nc.gpsimd
        nco = 256 - tree_co0
        if nco:
            T1 = treep.tile([128, 32 * nco], bf16, tag="tree1")
            T2 = treep.tile([128, 16 * nco], bf16, tag="tree2")
            T3 = treep.tile([128, 8 * nco], bf16, tag="tree3")
            co0 = tree_co0
            # T1[p] = M9[j=p] + M9[j=p+4], p=0..3 (last dims stride 1 -> 2x)
            nc.vector.tensor_add(
                AP(T1.tensor, 0, [[32 * nco, 128], [4 * nco, WLC], [4, nco], [1, 4]]),
                AP(M9.tensor, 9 * co0, [[M9F, 128], [2304, WLC], [9, nco], [1, 4]]),
                AP(M9.tensor, 9 * co0 + 4, [[M9F, 128], [2304, WLC], [9, nco], [1, 4]]),
            )
            nc.vector.tensor_add(
                AP(T2.tensor, 0, [[16 * nco, 128], [2 * nco, WLC], [2, nco], [1, 2]]),
                AP(T1.tensor, 0, [[32 * nco, 128], [4 * nco, WLC], [4, nco], [1, 2]]),
                AP(T1.tensor, 2, [[32 * nco, 128], [4 * nco, WLC], [4, nco], [1, 2]]),
            )
            t34eng.tensor_add(
                AP(T3.tensor, 0, [[8 * nco, 128], [nco, WLC], [1, nco]]),
                AP(T2.tensor, 0, [[16 * nco, 128], [2 * nco, WLC], [2, nco]]),
                AP(T2.tensor, 1, [[16 * nco, 128], [2 * nco, WLC], [2, nco]]),
            )
            t34eng.tensor_add(
                AP(outc.tensor, co0 * WLC,
                   [[WLC * 256, 128], [1, WLC], [WLC, nco]]),
                AP(T3.tensor, 0, [[8 * nco, 128], [nco, WLC], [1, nco]]),
                AP(M9.tensor, 9 * co0 + 8, [[M9F, 128], [2304, WLC], [9, nco]]),
            )

        # store chunk (SP); last chunk: co [0,192) as one store, then the
        # 16-co tail blocks individually right after each evac (last two on
        # Act so they don't queue behind SP's big store)
        if c == NCHUNK - 1:
            for eng, (s0, sn) in [(nc.sync, (0, 192)), (nc.sync, (192, 16)),
                                  (nc.sync, (208, 16)), (nc.scalar, (224, 16)),
                                  (nc.scalar, (240, 16))]:
                eng.dma_start(
                    out=AP(out_d, c * WLC * 256 + s0 * WLC,
                           [[32 * C, 128], [1, sn * WLC]]),
                    in_=AP(outc.tensor, s0 * WLC,
                           [[WLC * 256, 128], [1, sn * WLC]]),
                )
        else:
            nc.sync.dma_start(
                out=AP(out_d, c * WLC * 256, [[32 * C, 128], [1, WLC * 256]]),
                in_=AP(outc.tensor, 0, [[WLC * 256, 128], [1, WLC * 256]]),
            )


def _get_program(n_repeat=1, with_bias=True):
    key = ("nc", n_repeat, with_bias)
    if key not in _cache:
        _cache[key] = _build_program(n_repeat, with_bias)
    return _cache[key]


def _make_in_maps(inputs):
    x, W1, b1, W2, b2 = (inputs[k] for k in ("x", "W1", "b1", "W2", "b2"))
    bf = ml_dtypes.bfloat16
    Wc = (np.asarray(W1, np.float32) @ np.asarray(W2, np.float32))
    bc = (np.asarray(b1, np.float32) @ np.asarray(W2, np.float32)
          + np.asarray(b2, np.float32))
    wc_h = np.ascontiguousarray(Wc.astype(bf).reshape(2, 128, M144))
    bc_h = np.ascontiguousarray(bc.astype(bf).reshape(1, M144))
    xb = np.asarray(x).astype(bf)
    xp = np.zeros((NCORES, PW, PW, C), dtype=bf)
    xp[:, 1:65, 1:65, :] = xb
    # xT[i, half, c, (hl, wl, hb, wb)] = x[i, hb*4+hl, wb*8+wl, half*128+c]
    xt = (xb.reshape(NCORES, 16, 4, 8, 8, 2, 128)
          .transpose(0, 5, 6, 2, 4, 1, 3)      # i, half, c, hl, wl, hb, wb
          .reshape(NCORES, 2, 128, XTF))
    return [
        {
            "xpad": np.ascontiguousarray(xp[i]),
            "xt": np.ascontiguousarray(xt[i]),
            "wc": wc_h,
            "bc": bc_h,
        }
        for i in range(NCORES)
    ]


def _unpermute(raw):
    # [128, 32*256] p-major -> [64, 64, 256]: p = hb*8+wb, free = (hl, co, wl)
    return (np.asarray(raw).reshape(16, 8, 4, 256, 8)
            .transpose(0, 2, 1, 4, 3)
            .reshape(64, 64, 256))


def kernel(x, W1, b1, W2, b2, trace=False):
    from concourse.bass_utils import run_bass_kernel_spmd

    bc = np.asarray(b1, np.float32) @ np.asarray(W2, np.float32) + np.asarray(
        b2, np.float32)
    nc = _get_program(with_bias=bool(np.any(bc != 0.0)))
    in_maps = _make_in_maps(dict(x=x, W1=W1, b1=b1, W2=W2, b2=b2))
    res = run_bass_kernel_spmd(nc, in_maps, core_ids=list(range(NCORES)),
                               trace=trace)
    out = np.stack([_unpermute(res.results[i]["out"]) for i in range(NCORES)],
                   axis=0).astype(np.float32)
    if trace:
        return out, res
    return out


# revision 58
# speedup vs baseline: 1.0305x; 1.0305x over previous
# Trainium2 Bass kernel for nn_Invo2D, v7: host-transposed xT (no PE
# transposes), row-merged product pieces, DVE/Pool product split, PE/DVE fold
# split, DMA spread across engine queues, fine-grained last-chunk tail.
#
#   t2 = x @ Wc + bc     (Wc = W1@W2, bc = b1@W2+b2 host-folded, bf16)
#   out[pix, co] = sum_j t2[pix, 9*(co//16)+j] * x_tap(f)[pix, ch(f)], f = 9*co+j
#
# Per core (1 image): partition p = hb*8 + wb (16 h-blocks x 8 w-blocks);
# each partition owns a 4x8 pixel block with a 1-pixel 2D halo: free dim =
# (6 h-slots x 10 w-slots x 256 ch).  x zero-padded to [66,66,256] bf16 on
# host.  X flat addr for product f of chunk c at wl: c*2560 + row*1792 +
# wl*256 + f, where row = (f//768); pieces only cut at tap-row boundaries
# (f=768,1536) and group boundaries -> 22 pieces/chunk.
# t2 matmuls use a host-prepared transposed copy xT [2,128,4096] (ch-major),
# so PE does no transposes.  Fold: PE identity-lhsT PSUM-accumulate for co
# [0, TREE_CO0), DVE/Pool bf16 add-tree for the rest; Act evacuates PSUM.
import numpy as np
import ml_dtypes

H, W, C = 64, 64, 256
M144 = 144
NCORES = 8
HS, WS = 6, 10          # h-slots, w-slots per partition (4+2, 8+2)
XF = HS * WS * C        # 15360 bf16 per partition
WLC = 8                 # wl positions per chunk = one hl row
NCHUNK = 4
M9F = WLC * 2304        # 18432
W9F = WLC * M144        # 1152
PW = 66                 # padded row length
XTF = 4096              # pixels per xT half

# fold split: PE folds co [0, TREE_CO0[c]) in 64-blocks, DVE tree the rest.
# Last chunk: PE tail in 16-co blocks with per-block stores and an all-DVE
# tree for the final co range so the tail chain has no cross-engine sems.
TREE_CO0 = (208, 208, 192, 224)
# product piece routing: relative rates (ns per free elem) + per-instr cost
DVE_NS = 0.52
POOL_NS = 0.833
DVE_INSTR_NS = 60.0
ROUTE_TD0_BIAS = 150.0   # empirically tuned DVE seed bias (sim sweep)

_cache = {}


# groups 5 and 10 straddle tap-row boundaries; their rect pieces
# (g, gc0, ngc, j0, nj, row):
_PARTIALS = {
    5: [(5, 0, 5, 0, 9, 0), (5, 5, 1, 0, 3, 0),
        (5, 5, 1, 3, 6, 1), (5, 6, 10, 0, 9, 1)],
    10: [(10, 0, 10, 0, 9, 1), (10, 10, 1, 0, 6, 1),
         (10, 10, 1, 6, 3, 2), (10, 11, 5, 0, 9, 2)],
}
_ROW_OF_FULL = {g: (0 if g < 5 else (1 if g < 10 else 2)) for g in range(16)
                if g not in (5, 10)}

def _build_pieces():
    """Pieces (g, gc0, ngc, j0, nj, row) in f order: full groups stay whole;
    groups 5 and 10 split into rects at tap-row boundaries."""
    pieces = []
    for g in range(16):
        if g in _PARTIALS:
            pieces.extend(_PARTIALS[g])
        else:
            pieces.append((g, 0, 16, 0, 9, _ROW_OF_FULL[g]))
    return pieces


def _route_pieces(pieces, td0=0.0, tp0=0.0):
    """Greedy DVE/Pool routing: assign each piece (f-ascending) to the engine
    with the lower projected finish time.  td0/tp0 seed the per-chunk fold
    burden each engine already carries (DVE: T1+T2 tree, Pool: T3+final).
    DVE pays a per-instruction SBUF-access cost; Pool does not (v1 model)."""
    td, tp = td0, tp0
    routed = []
    for pc in pieces:
        fe = WLC * pc[2] * pc[4]
        if td + fe * DVE_NS + DVE_INSTR_NS <= tp + fe * POOL_NS:
            td += fe * DVE_NS + DVE_INSTR_NS
            routed.append(("dve",) + pc)
        else:
            tp += fe * POOL_NS
            routed.append(("pool",) + pc)
    return routed


def _build_program(n_repeat=1, with_bias=True):
    import concourse.bass as bass
    import concourse.tile as tile
    from concourse import bacc, mybir
    from concourse.masks import make_identity

    f32 = mybir.dt.float32
    bf16 = mybir.dt.bfloat16
    AP = bass.AP

    nc = bacc.Bacc(None, target_bir_lowering=False)
    x_d = nc.dram_tensor("xpad", [PW, PW, C], bf16, kind="ExternalInput")
    xt_d = nc.dram_tensor("xt", [2, 128, XTF], bf16, kind="ExternalInput")
    wc_d = nc.dram_tensor("wc", [2, 128, M144], bf16, kind="ExternalInput")
    bc_d = nc.dram_tensor("bc", [1, M144], bf16, kind="ExternalInput")
    out_d = nc.dram_tensor("out", [128, 32 * C], bf16, kind="ExternalOutput")

    pieces = _build_pieces()
    ROUTED = []
    for c in range(NCHUNK):
        nco_tree = 256 - TREE_CO0[c]
        td0 = (32 + 16) * nco_tree * DVE_NS + ROUTE_TD0_BIAS
        tp0 = (8 + 8) * nco_tree * POOL_NS        # T3 + final on Pool
        ROUTED.append(_route_pieces(pieces, td0, tp0))

    with tile.TileContext(nc) as tc:
        with (
            tc.tile_pool(name="singles", bufs=1) as singles,
            tc.tile_pool(name="xbufs", bufs=1) as xbufs,
            tc.tile_pool(name="w9p", bufs=1) as w9p,
            tc.tile_pool(name="m9p", bufs=3) as m9p,
            tc.tile_pool(name="treep", bufs=2) as treep,
            tc.tile_pool(name="outp", bufs=2) as outp,
            tc.tile_pool(name="ps2p", bufs=3, space="PSUM") as ps2p,
            tc.tile_pool(name="foldp", bufs=3, space="PSUM") as foldp,
        ):
            ident = singles.tile([128, 128], bf16)
            wc01 = singles.tile([128, 2 * M144], bf16)
            bcsb = singles.tile([1, M144], bf16)
            ones1 = singles.tile([1, 128], bf16)
            XT0 = singles.tile([128, XTF], bf16)
            XT1 = singles.tile([128, XTF], bf16)
            W9c = [w9p.tile([128, W9F], bf16, name=f"w9_{c}", tag=f"w9_{c}")
                   for c in range(NCHUNK)]

            for _rep in range(n_repeat):
                _image_body(nc, bass, mybir, make_identity,
                            xbufs, m9p, treep, outp, ps2p, foldp,
                            ident, wc01, bcsb, ones1, XT0, XT1, W9c,
                            x_d, xt_d, wc_d, bc_d, out_d, ROUTED, with_bias)
    nc.compile()
    return nc


def _image_body(nc, bass, mybir, make_identity,
                xbufs, m9p, treep, outp, ps2p, foldp,
                ident, wc01, bcsb, ones1, XT0, XT1, W9c,
                x_d, xt_d, wc_d, bc_d, out_d, ROUTED, with_bias):
    f32 = mybir.dt.float32
    bf16 = mybir.dt.bfloat16
    AP = bass.AP

    X = xbufs.tile([128, XF], bf16, tag="X")
    XTs = (XT0, XT1)

    def xt_load(eng, h, hl0, nhl):
        eng.dma_start(
            out=AP(XTs[h].tensor, hl0 * 1024, [[XTF, 128], [1, nhl * 1024]]),
            in_=AP(xt_d, h * 128 * XTF + hl0 * 1024,
                   [[XTF, 128], [1, nhl * 1024]]),
        )

    def x_slice(eng, s):
        eng.dma_start(
            out=AP(X.tensor, s * WS * C, [[XF, 128], [1, WS * C]]),
            in_=AP(x_d, s * PW * C,
                   [[4 * PW * C, 16], [8 * C, 8], [1, WS * C]]),
        )

    # -- prologue DMAs spread across queues (SP, Act, Pool) --
    # chunk-c row-r product pieces read only hslot c+r, so for chunk 0 the
    # slice deadlines relax: s0 ~4us, s1 ~5.5us, s2 ~7us.  Act's queue is
    # [wc0, wc1, <auto ATL>, evacs] so the act-table load overlaps t2.
    xt_load(nc.sync, 0, 0, 1)            # SP head: xT half0 row0
    xt_load(nc.scalar, 1, 0, 1)          # Act head: xT half1 row0 (parallel)
    # both wc halves in one DMA on Pool (delivery ~2.4us < first matmul)
    nc.gpsimd.dma_start(
        out=wc01[:],
        in_=AP(wc_d, 0, [[M144, 128], [128 * M144, 2], [1, M144]]),
    )
    if with_bias:
        nc.scalar.dma_start(out=bcsb[:], in_=bc_d[:])
        nc.vector.memset(ones1[:], 1.0)
    make_identity(nc, ident[:])          # Pool, ~0.2us
    x_slice(nc.sync, 0)
    x_slice(nc.sync, 1)
    xt_load(nc.sync, 0, 1, 1)
    xt_load(nc.sync, 1, 1, 1)
    x_slice(nc.sync, 3)
    xt_load(nc.sync, 0, 2, 1)
    xt_load(nc.sync, 1, 2, 1)
    x_slice(nc.sync, 4)
    xt_load(nc.sync, 0, 3, 1)
    xt_load(nc.sync, 1, 3, 1)
    x_slice(nc.sync, 5)

    def t2_grp(r, split_evac=False):
        # 4 pair-tiles; 2 tiles share one PSUM tile, evacuated 288 wide.
        # grp0: evac pairs 0,1 on Act and 2,3 on DVE in parallel (W9c[0]
        # gates all products; DVE idles until then anyway).
        for q in range(4):
            ps2 = ps2p.tile([128, 288], f32, padded_shape=[128, 512])
            for i in range(2):
                t = r * 8 + 2 * q + i
                dst = ps2[:, i * M144:(i + 1) * M144]
                nc.tensor.matmul(dst, lhsT=AP(XT0.tensor, t * 128,
                                              [[XTF, 128], [1, 128]]),
                                 rhs=wc01[:, 0:M144], start=True, stop=False)
                nc.tensor.matmul(dst, lhsT=AP(XT1.tensor, t * 128,
                                              [[XTF, 128], [1, 128]]),
                                 rhs=wc01[:, M144:2 * M144], start=False,
                                 stop=not with_bias)
                if with_bias:
                    nc.tensor.matmul(dst, lhsT=ones1[:], rhs=bcsb[:],
                                     start=False, stop=True)
            w9dst = AP(W9c[r].tensor, 2 * q * M144, [[W9F, 128], [1, 288]])
            if split_evac and q % 2 == 1:
                nc.vector.tensor_copy(out=w9dst, in_=ps2[:])
            else:
                nc.scalar.copy(out=w9dst, in_=ps2[:])

    t2_grp(0, split_evac=True)
    # Act then loads x slice 2 (needed by chunk-0 row-2 products ~8us)
    x_slice(nc.scalar, 2)
    t2_grp(1)

    for c in range(NCHUNK):
        M9 = m9p.tile([128, M9F], bf16, tag="m9")
        # products, f-ascending, routed DVE/Pool.  Chunk 0's first pieces are
        # split into wl halves: the wl 0-3 half only reads W9 pairs 0-1, so
        # products start before the last grp-0 evacuations land.
        for (eng_name, g, gc0, ngc, j0, nj, row) in ROUTED[c]:
            base = c * 2560 + row * 1792
            f0 = 144 * g + 9 * gc0 + j0
            eng = nc.vector if eng_name == "dve" else nc.gpsimd
            halves = ((0, 4), (4, 4)) if (c == 0 and g <= 2) else ((0, WLC),)
            for wl0, nwl in halves:
                eng.tensor_mul(
                    AP(M9.tensor, wl0 * 2304 + f0,
                       [[M9F, 128], [2304, nwl], [9, ngc], [1, nj]]),
                    AP(X.tensor, base + wl0 * 256 + f0,
                       [[XF, 128], [256, nwl], [9, ngc], [1, nj]]),
                    AP(W9c[c].tensor, wl0 * M144 + 9 * g + j0,
                       [[W9F, 128], [M144, nwl], [0, ngc], [1, nj]]),
                )

        outc = outp.tile([128, WLC * 256], bf16, tag="outc")
        tree_co0 = TREE_CO0[c]

        # PE fold blocks for co [0, tree_co0); last chunk uses fine blocks so
        # PE tracks the product stream and the tail chain is short
        pe_blocks = []
        if c == NCHUNK - 1:
            pe_blocks += [(b, 32) for b in range(0, tree_co0, 32)]
        else:
            for co0 in range(0, tree_co0, 64):
                pe_blocks.append((co0, min(64, tree_co0 - co0)))
        for co0, nco in pe_blocks:
            ps = foldp.tile([128, 512], f32, padded_shape=[128, 512])
            for j in range(9):
                nc.tensor.matmul(
                    ps[:, :WLC * nco],
                    lhsT=ident[:],
                    rhs=AP(M9.tensor, 9 * co0 + j,
                           [[M9F, 128], [2304, WLC], [9, nco]]),
                    start=(j == 0),
                    stop=(j == 8),
                )
            # outc is co-major: outc[p, co*8 + wl]
            nc.scalar.copy(
                out=AP(outc.tensor, co0 * WLC,
                       [[WLC * 256, 128], [1, WLC], [WLC, nco]]),
                in_=AP(ps.tensor, 0, [[512, 128], [1, nco * WLC]]),
            )

        # interleave t2 for the next-next group while folds run
        if c + 2 < NCHUNK:
            t2_grp(c + 2)

        # DVE/Pool add-tree for co [tree_co0, 256).  Normal chunks: T1/T2 on
        # DVE, T3/final on Pool.  Last chunk: TWO independent half-trees run
        # in parallel (DVE and Pool), halving the serial tail fold.
        last = c == NCHUNK - 1
        def add_tree(co0, nco, tag, e12, e34):
            T1 = treep.tile([128, 32 * nco], bf16, tag=f"tree1{tag}")
            T2 = treep.tile([128, 16 * nco], bf16, tag=f"tree2{tag}")
            T3 = treep.tile([128, 8 * nco], bf16, tag=f"tree3{tag}")
            # T1[p] = M9[j=p] + M9[j=p+4], p=0..3 (last dims stride 1 -> 2x)
            e12.tensor_add(
                AP(T1.tensor, 0, [[32 * nco, 128], [4 * nco, WLC], [4, nco], [1, 4]]),
                AP(M9.tensor, 9 * co0, [[M9F, 128], [2304, WLC], [9, nco], [1, 4]]),
                AP(M9.tensor, 9 * co0 + 4, [[M9F, 128], [2304, WLC], [9, nco], [1, 4]]),
            )
            e12.tensor_add(
                AP(T2.tensor, 0, [[16 * nco, 128], [2 * nco, WLC], [2, nco], [1, 2]]),
                AP(T1.tensor, 0, [[32 * nco, 128], [4 * nco, WLC], [4, nco], [1, 2]]),
                AP(T1.tensor, 2, [[32 * nco, 128], [4 * nco, WLC], [4, nco], [1, 2]]),
            )
            e34.tensor_add(
                AP(T3.tensor, 0, [[8 * nco, 128], [nco, WLC], [1, nco]]),
                AP(T2.tensor, 0, [[16 * nco, 128], [2 * nco, WLC], [2, nco]]),
                AP(T2.tensor, 1, [[16 * nco, 128], [2 * nco, WLC], [2, nco]]),
            )
            e34.tensor_add(
                AP(outc.tensor, co0 * WLC,
                   [[WLC * 256, 128], [1, WLC], [WLC, nco]]),
                AP(T3.tensor, 0, [[8 * nco, 128], [nco, WLC], [1, nco]]),
                AP(M9.tensor, 9 * co0 + 8, [[M9F, 128], [2304, WLC], [9, nco]]),
            )
        nco = 256 - tree_co0
        if nco:
            if last and nco >= 32:
                half = nco // 2
                add_tree(tree_co0, half, "a", nc.vector, nc.vector)
                add_tree(tree_co0 + half, nco - half, "b", nc.gpsimd,
                         nc.gpsimd)
            else:
                add_tree(tree_co0, nco, "", nc.vector, nc.gpsimd)

        # store chunk (SP); last chunk: co [0,192) as one store, then the
        # 16-co tail blocks individually right after each evac (last two on
        # Act so they don't queue behind SP's big store)
        if c == NCHUNK - 1:
            for eng, (s0, sn) in [(nc.sync, (0, 128)), (nc.sync, (128, 64)),
                                  (nc.scalar, (192, 32)), (nc.sync, (224, 16)),
                                  (nc.sync, (240, 16))]:
                eng.dma_start(
                    out=AP(out_d, c * WLC * 256 + s0 * WLC,
                           [[32 * C, 128], [1, sn * WLC]]),
                    in_=AP(outc.tensor, s0 * WLC,
                           [[WLC * 256, 128], [1, sn * WLC]]),
                )
        else:
            nc.sync.dma_start(
                out=AP(out_d, c * WLC * 256, [[32 * C, 128], [1, WLC * 256]]),
                in_=AP(outc.tensor, 0, [[WLC * 256, 128], [1, WLC * 256]]),
            )


def _get_program(n_repeat=1, with_bias=True):
    key = ("nc", n_repeat, with_bias)
    if key not in _cache:
        _cache[key] = _build_program(n_repeat, with_bias)
    return _cache[key]


def _make_in_maps(inputs):
    x, W1, b1, W2, b2 = (inputs[k] for k in ("x", "W1", "b1", "W2", "b2"))
    bf = ml_dtypes.bfloat16
    Wc = (np.asarray(W1, np.float32) @ np.asarray(W2, np.float32))
    bc = (np.asarray(b1, np.float32) @ np.asarray(W2, np.float32)
          + np.asarray(b2, np.float32))
    wc_h = np.ascontiguousarray(Wc.astype(bf).reshape(2, 128, M144))
    bc_h = np.ascontiguousarray(bc.astype(bf).reshape(1, M144))
    xb = np.asarray(x).astype(bf)
    xp = np.zeros((NCORES, PW, PW, C), dtype=bf)
    xp[:, 1:65, 1:65, :] = xb
    # xT[i, half, c, (hl, wl, hb, wb)] = x[i, hb*4+hl, wb*8+wl, half*128+c]
    xt = (xb.reshape(NCORES, 16, 4, 8, 8, 2, 128)
          .transpose(0, 5, 6, 2, 4, 1, 3)      # i, half, c, hl, wl, hb, wb
          .reshape(NCORES, 2, 128, XTF))
    return [
        {
            "xpad": np.ascontiguousarray(xp[i]),
            "xt": np.ascontiguousarray(xt[i]),
            "wc": wc_h,
            "bc": bc_h,
        }
        for i in range(NCORES)
    ]


def _unpermute(raw):
    # [128, 32*256] p-major -> [64, 64, 256]: p = hb*8+wb, free = (hl, co, wl)
    return (np.asarray(raw).reshape(16, 8, 4, 256, 8)
            .transpose(0, 2, 1, 4, 3)
            .reshape(64, 64, 256))


def kernel(x, W1, b1, W2, b2, trace=False):
    from concourse.bass_utils import run_bass_kernel_spmd

    bc = np.asarray(b1, np.float32) @ np.asarray(W2, np.float32) + np.asarray(
        b2, np.float32)
    nc = _get_program(with_bias=bool(np.any(bc != 0.0)))
    in_maps = _make_in_maps(dict(x=x, W1=W1, b1=b1, W2=W2, b2=b2))
    res = run_bass_kernel_spmd(nc, in_maps, core_ids=list(range(NCORES)),
                               trace=trace)
    out = np.stack([_unpermute(res.results[i]["out"]) for i in range(NCORES)],
                   axis=0).astype(np.float32)
    if trace:
        return out, res
    return out


# revision 74
# speedup vs baseline: 1.0338x; 1.0032x over previous
# Trainium2 Bass kernel for nn_Invo2D, v7: host-transposed xT (no PE
# transposes), row-merged product pieces, DVE/Pool product split, PE/DVE fold
# split, DMA spread across engine queues, fine-grained last-chunk tail.
#
#   t2 = x @ Wc + bc     (Wc = W1@W2, bc = b1@W2+b2 host-folded, bf16)
#   out[pix, co] = sum_j t2[pix, 9*(co//16)+j] * x_tap(f)[pix, ch(f)], f = 9*co+j
#
# Per core (1 image): partition p = hb*8 + wb (16 h-blocks x 8 w-blocks);
# each partition owns a 4x8 pixel block with a 1-pixel 2D halo: free dim =
# (6 h-slots x 10 w-slots x 256 ch).  x zero-padded to [66,66,256] bf16 on
# host.  X flat addr for product f of chunk c at wl: c*2560 + row*1792 +
# wl*256 + f, where row = (f//768); pieces only cut at tap-row boundaries
# (f=768,1536) and group boundaries -> 22 pieces/chunk.
# t2 matmuls use a host-prepared transposed copy xT [2,128,4096] (ch-major),
# so PE does no transposes.  Fold: PE identity-lhsT PSUM-accumulate for co
# [0, TREE_CO0), DVE/Pool bf16 add-tree for the rest; Act evacuates PSUM.
import numpy as np
import ml_dtypes

H, W, C = 64, 64, 256
M144 = 144
NCORES = 8
HS, WS = 6, 10          # h-slots, w-slots per partition (4+2, 8+2)
XF = HS * WS * C        # 15360 bf16 per partition
WLC = 8                 # wl positions per chunk = one hl row
NCHUNK = 4
M9F = WLC * 2304        # 18432
W9F = WLC * M144        # 1152
PW = 66                 # padded row length
XTF = 4096              # pixels per xT half

# fold split: PE folds co [0, TREE_CO0[c]) in 64-blocks, DVE tree the rest.
# Last chunk: PE tail in 16-co blocks with per-block stores and an all-DVE
# tree for the final co range so the tail chain has no cross-engine sems.
TREE_CO0 = (208, 208, 192, 224)
# product piece routing: relative rates (ns per free elem) + per-instr cost
DVE_NS = 0.52
POOL_NS = 0.833
DVE_INSTR_NS = 60.0
ROUTE_TD0_BIAS = 150.0   # empirically tuned DVE seed bias (sim sweep)

_cache = {}


# groups 5 and 10 straddle tap-row boundaries; their rect pieces
# (g, gc0, ngc, j0, nj, row):
_PARTIALS = {
    5: [(5, 0, 5, 0, 9, 0), (5, 5, 1, 0, 3, 0),
        (5, 5, 1, 3, 6, 1), (5, 6, 10, 0, 9, 1)],
    10: [(10, 0, 10, 0, 9, 1), (10, 10, 1, 0, 6, 1),
         (10, 10, 1, 6, 3, 2), (10, 11, 5, 0, 9, 2)],
}
_ROW_OF_FULL = {g: (0 if g < 5 else (1 if g < 10 else 2)) for g in range(16)
                if g not in (5, 10)}

def _build_pieces():
    """Pieces (g, gc0, ngc, j0, nj, row) in f order: full groups stay whole;
    groups 5 and 10 split into rects at tap-row boundaries."""
    pieces = []
    for g in range(16):
        if g in _PARTIALS:
            pieces.extend(_PARTIALS[g])
        else:
            pieces.append((g, 0, 16, 0, 9, _ROW_OF_FULL[g]))
    return pieces


def _route_pieces(pieces, td0=0.0, tp0=0.0):
    """Greedy DVE/Pool routing: assign each piece (f-ascending) to the engine
    with the lower projected finish time.  td0/tp0 seed the per-chunk fold
    burden each engine already carries (DVE: T1+T2 tree, Pool: T3+final).
    DVE pays a per-instruction SBUF-access cost; Pool does not (v1 model)."""
    td, tp = td0, tp0
    routed = []
    for pc in pieces:
        fe = WLC * pc[2] * pc[4]
        if td + fe * DVE_NS + DVE_INSTR_NS <= tp + fe * POOL_NS:
            td += fe * DVE_NS + DVE_INSTR_NS
            routed.append(("dve",) + pc)
        else:
            tp += fe * POOL_NS
            routed.append(("pool",) + pc)
    return routed


def _merge_dve_runs(routed, protect_g=-1):
    """Merge consecutive full-group DVE pieces with consecutive g, same tap
    row, AND the same fold block (g//4) — such merges cannot delay any PE
    fold block.  Merged entries get ng>1 (emitted with a 5-dim W9 AP)."""
    out = []
    for (eng, g, gc0, ngc, j0, nj, row) in routed:
        full = gc0 == 0 and ngc == 16 and j0 == 0 and nj == 9
        if (out and eng == "dve" and full and g > protect_g):
            peng, pg, png, pgc0, pngc, pj0, pnj, prow = out[-1]
            if (peng == "dve" and pgc0 == 0 and pngc == 16 and pnj == 9
                    and pg + png == g and prow == row
                    and pg // 4 == g // 4 and pg > protect_g):
                out[-1] = (peng, pg, png + 1, 0, 16, 0, 9, row)
                continue
        out.append((eng, g, 1, gc0, ngc, j0, nj, row))
    return out


def _build_program(n_repeat=1, with_bias=True):
    import concourse.bass as bass
    import concourse.tile as tile
    from concourse import bacc, mybir
    from concourse.masks import make_identity

    f32 = mybir.dt.float32
    bf16 = mybir.dt.bfloat16
    AP = bass.AP

    nc = bacc.Bacc(None, target_bir_lowering=False)
    x_d = nc.dram_tensor("xpad", [PW, PW, C], bf16, kind="ExternalInput")
    xt_d = nc.dram_tensor("xt", [2, 128, XTF], bf16, kind="ExternalInput")
    wc_d = nc.dram_tensor("wc", [2, 128, M144], bf16, kind="ExternalInput")
    bc_d = nc.dram_tensor("bc", [1, M144], bf16, kind="ExternalInput")
    out_d = nc.dram_tensor("out", [128, 32 * C], bf16, kind="ExternalOutput")

    pieces = _build_pieces()
    ROUTED = []
    for c in range(NCHUNK):
        nco_tree = 256 - TREE_CO0[c]
        td0 = (32 + 16) * nco_tree * DVE_NS + ROUTE_TD0_BIAS
        tp0 = (8 + 8) * nco_tree * POOL_NS        # T3 + final on Pool
        routed = _route_pieces(pieces, td0, tp0)
        # NOTE: _merge_dve_runs' 5-dim W9 APs pass CoreSim but neuronxcc's
        # codegen only supports TENSOR3D (3 free dims) for tensor ops --
        # keep pieces unmerged (measured equal anyway).
        ROUTED.append([(e, g, 1, gc0, ngc, j0, nj, row)
                       for (e, g, gc0, ngc, j0, nj, row) in routed])

    with tile.TileContext(nc) as tc:
        with (
            tc.tile_pool(name="singles", bufs=1) as singles,
            tc.tile_pool(name="xbufs", bufs=1) as xbufs,
            tc.tile_pool(name="w9p", bufs=1) as w9p,
            tc.tile_pool(name="m9p", bufs=3) as m9p,
            tc.tile_pool(name="treep", bufs=2) as treep,
            tc.tile_pool(name="outp", bufs=2) as outp,
            tc.tile_pool(name="ps2p", bufs=3, space="PSUM") as ps2p,
            tc.tile_pool(name="foldp", bufs=3, space="PSUM") as foldp,
        ):
            ident = singles.tile([128, 128], bf16)
            wc01 = singles.tile([128, 2 * M144], bf16)
            bcsb = singles.tile([1, M144], bf16)
            ones1 = singles.tile([1, 128], bf16)
            XT0 = singles.tile([128, XTF], bf16)
            XT1 = singles.tile([128, XTF], bf16)
            W9c = [w9p.tile([128, W9F], bf16, name=f"w9_{c}", tag=f"w9_{c}")
                   for c in range(NCHUNK)]

            for _rep in range(n_repeat):
                _image_body(nc, bass, mybir, make_identity,
                            xbufs, m9p, treep, outp, ps2p, foldp,
                            ident, wc01, bcsb, ones1, XT0, XT1, W9c,
                            x_d, xt_d, wc_d, bc_d, out_d, ROUTED, with_bias)
    nc.compile()
    return nc


def _image_body(nc, bass, mybir, make_identity,
                xbufs, m9p, treep, outp, ps2p, foldp,
                ident, wc01, bcsb, ones1, XT0, XT1, W9c,
                x_d, xt_d, wc_d, bc_d, out_d, ROUTED, with_bias):
    f32 = mybir.dt.float32
    bf16 = mybir.dt.bfloat16
    AP = bass.AP

    X = xbufs.tile([128, XF], bf16, tag="X")
    XTs = (XT0, XT1)

    def xt_load(eng, h, hl0, nhl, half=None):
        off, n = hl0 * 1024, nhl * 1024
        if half is not None:
            off, n = off + half * 512, 512
        eng.dma_start(
            out=AP(XTs[h].tensor, off, [[XTF, 128], [1, n]]),
            in_=AP(xt_d, h * 128 * XTF + off, [[XTF, 128], [1, n]]),
        )

    def x_slice(eng, s):
        eng.dma_start(
            out=AP(X.tensor, s * WS * C, [[XF, 128], [1, WS * C]]),
            in_=AP(x_d, s * PW * C,
                   [[4 * PW * C, 16], [8 * C, 8], [1, WS * C]]),
        )

    # -- prologue DMAs spread across queues (SP, Act, Pool) --
    # chunk-c row-r product pieces read only hslot c+r, so for chunk 0 the
    # slice deadlines relax: s0 ~4us, s1 ~5.5us, s2 ~7us.  Act's queue is
    # [wc0, wc1, <auto ATL>, evacs] so the act-table load overlaps t2.
    xt_load(nc.sync, 0, 0, 1)            # SP head: xT half0 row0
    xt_load(nc.scalar, 1, 0, 1)          # Act head: xT half1 row0 (parallel)
    # both wc halves in one DMA on Pool (delivery ~2.4us < first matmul)
    nc.gpsimd.dma_start(
        out=wc01[:],
        in_=AP(wc_d, 0, [[M144, 128], [128 * M144, 2], [1, M144]]),
    )
    if with_bias:
        nc.scalar.dma_start(out=bcsb[:], in_=bc_d[:])
        nc.vector.memset(ones1[:], 1.0)
    make_identity(nc, ident[:])          # Pool, ~0.2us
    x_slice(nc.sync, 0)
    x_slice(nc.gpsimd, 1)                # Pool DMA stream, parallel to SP
    xt_load(nc.sync, 0, 1, 1)
    xt_load(nc.sync, 1, 1, 1)
    x_slice(nc.gpsimd, 3)
    xt_load(nc.sync, 0, 2, 1)
    xt_load(nc.sync, 1, 2, 1)
    x_slice(nc.sync, 4)
    xt_load(nc.sync, 0, 3, 1)
    xt_load(nc.sync, 1, 3, 1)
    x_slice(nc.sync, 5)

    def t2_grp(r, split_evac=False):
        # 4 pair-tiles; 2 tiles share one PSUM tile, evacuated 288 wide.
        # grp0: evac pairs 0,1 on Act and 2,3 on DVE in parallel (W9c[0]
        # gates all products; DVE idles until then anyway).
        for q in range(4):
            ps2 = ps2p.tile([128, 288], f32, padded_shape=[128, 512])
            for i in range(2):
                t = r * 8 + 2 * q + i
                dst = ps2[:, i * M144:(i + 1) * M144]
                nc.tensor.matmul(dst, lhsT=AP(XT0.tensor, t * 128,
                                              [[XTF, 128], [1, 128]]),
                                 rhs=wc01[:, 0:M144], start=True, stop=False)
                nc.tensor.matmul(dst, lhsT=AP(XT1.tensor, t * 128,
                                              [[XTF, 128], [1, 128]]),
                                 rhs=wc01[:, M144:2 * M144], start=False,
                                 stop=not with_bias)
                if with_bias:
                    nc.tensor.matmul(dst, lhsT=ones1[:], rhs=bcsb[:],
                                     start=False, stop=True)
            w9dst = AP(W9c[r].tensor, 2 * q * M144, [[W9F, 128], [1, 288]])
            if split_evac and q % 2 == 1:
                nc.vector.tensor_copy(out=w9dst, in_=ps2[:])
            else:
                nc.scalar.copy(out=w9dst, in_=ps2[:])

    t2_grp(0, split_evac=True)
    # Act then loads x slice 2 (needed by chunk-0 row-2 products ~8us)
    x_slice(nc.scalar, 2)
    t2_grp(1)

    for c in range(NCHUNK):
        M9 = m9p.tile([128, M9F], bf16, tag="m9")
        # products, f-ascending, routed DVE/Pool.  Chunk 0's first pieces are
        # split into wl halves: the wl 0-3 half only reads W9 pairs 0-1, so
        # products start before the last grp-0 evacuations land.
        for (eng_name, g, ng, gc0, ngc, j0, nj, row) in ROUTED[c]:
            base = c * 2560 + row * 1792
            f0 = 144 * g + 9 * gc0 + j0
            eng = nc.vector if eng_name == "dve" else nc.gpsimd
            if ng > 1:
                # merged full-group DVE run: f-contiguous X/M9, 5-dim W9
                eng.tensor_mul(
                    AP(M9.tensor, f0, [[M9F, 128], [2304, WLC], [1, 144 * ng]]),
                    AP(X.tensor, base + f0,
                       [[XF, 128], [256, WLC], [1, 144 * ng]]),
                    AP(W9c[c].tensor, 9 * g,
                       [[W9F, 128], [M144, WLC], [9, ng], [0, 16], [1, 9]]),
                )
                continue
            halves = ((0, 4), (4, 4)) if (c == 0 and g <= 2) else ((0, WLC),)
            for wl0, nwl in halves:
                eng.tensor_mul(
                    AP(M9.tensor, wl0 * 2304 + f0,
                       [[M9F, 128], [2304, nwl], [9, ngc], [1, nj]]),
                    AP(X.tensor, base + wl0 * 256 + f0,
                       [[XF, 128], [256, nwl], [9, ngc], [1, nj]]),
                    AP(W9c[c].tensor, wl0 * M144 + 9 * g + j0,
                       [[W9F, 128], [M144, nwl], [0, ngc], [1, nj]]),
                )

        outc = outp.tile([128, WLC * 256], bf16, tag="outc")
        tree_co0 = TREE_CO0[c]

        # PE fold blocks for co [0, tree_co0); last chunk uses fine blocks so
        # PE tracks the product stream and the tail chain is short
        pe_blocks = []
        if c == NCHUNK - 1:
            pe_blocks += [(b, 32) for b in range(0, tree_co0, 32)]
        else:
            for co0 in range(0, tree_co0, 64):
                pe_blocks.append((co0, min(64, tree_co0 - co0)))
        for co0, nco in pe_blocks:
            ps = foldp.tile([128, 512], f32, padded_shape=[128, 512])
            for j in range(9):
                nc.tensor.matmul(
                    ps[:, :WLC * nco],
                    lhsT=ident[:],
                    rhs=AP(M9.tensor, 9 * co0 + j,
                           [[M9F, 128], [2304, WLC], [9, nco]]),
                    start=(j == 0),
                    stop=(j == 8),
                )
            # outc is co-major: outc[p, co*8 + wl]
            nc.scalar.copy(
                out=AP(outc.tensor, co0 * WLC,
                       [[WLC * 256, 128], [1, WLC], [WLC, nco]]),
                in_=AP(ps.tensor, 0, [[512, 128], [1, nco * WLC]]),
            )

        # interleave t2 for the next-next group while folds run
        if c + 2 < NCHUNK:
            t2_grp(c + 2)

        # DVE/Pool add-tree for co [tree_co0, 256).  Normal chunks: T1/T2 on
        # DVE, T3/final on Pool.  Last chunk: TWO independent half-trees run
        # in parallel (DVE and Pool), halving the serial tail fold.
        last = c == NCHUNK - 1
        def add_tree(co0, nco, tag, e12, e34):
            T1 = treep.tile([128, 32 * nco], bf16, tag=f"tree1{tag}")
            T2 = treep.tile([128, 16 * nco], bf16, tag=f"tree2{tag}")
            T3 = treep.tile([128, 8 * nco], bf16, tag=f"tree3{tag}")
            # T1[p] = M9[j=p] + M9[j=p+4], p=0..3 (last dims stride 1 -> 2x)
            e12.tensor_add(
                AP(T1.tensor, 0, [[32 * nco, 128], [4 * nco, WLC], [4, nco], [1, 4]]),
                AP(M9.tensor, 9 * co0, [[M9F, 128], [2304, WLC], [9, nco], [1, 4]]),
                AP(M9.tensor, 9 * co0 + 4, [[M9F, 128], [2304, WLC], [9, nco], [1, 4]]),
            )
            e12.tensor_add(
                AP(T2.tensor, 0, [[16 * nco, 128], [2 * nco, WLC], [2, nco], [1, 2]]),
                AP(T1.tensor, 0, [[32 * nco, 128], [4 * nco, WLC], [4, nco], [1, 2]]),
                AP(T1.tensor, 2, [[32 * nco, 128], [4 * nco, WLC], [4, nco], [1, 2]]),
            )
            e34.tensor_add(
                AP(T3.tensor, 0, [[8 * nco, 128], [nco, WLC], [1, nco]]),
                AP(T2.tensor, 0, [[16 * nco, 128], [2 * nco, WLC], [2, nco]]),
                AP(T2.tensor, 1, [[16 * nco, 128], [2 * nco, WLC], [2, nco]]),
            )
            e34.tensor_add(
                AP(outc.tensor, co0 * WLC,
                   [[WLC * 256, 128], [1, WLC], [WLC, nco]]),
                AP(T3.tensor, 0, [[8 * nco, 128], [nco, WLC], [1, nco]]),
                AP(M9.tensor, 9 * co0 + 8, [[M9F, 128], [2304, WLC], [9, nco]]),
            )
        nco = 256 - tree_co0
        if nco:
            if last and nco >= 32:
                half = nco // 2
                add_tree(tree_co0, half, "a", nc.vector, nc.vector)
                add_tree(tree_co0 + half, nco - half, "b", nc.gpsimd,
                         nc.gpsimd)
            else:
                add_tree(tree_co0, nco, "", nc.vector, nc.gpsimd)

        # store chunk (SP); last chunk: co [0,192) as one store, then the
        # 16-co tail blocks individually right after each evac (last two on
        # Act so they don't queue behind SP's big store)
        if c == NCHUNK - 1:
            for eng, (s0, sn) in [(nc.sync, (0, 128)), (nc.sync, (128, 64)),
                                  (nc.scalar, (192, 32)), (nc.sync, (224, 32))]:
                eng.dma_start(
                    out=AP(out_d, c * WLC * 256 + s0 * WLC,
                           [[32 * C, 128], [1, sn * WLC]]),
                    in_=AP(outc.tensor, s0 * WLC,
                           [[WLC * 256, 128], [1, sn * WLC]]),
                )
        else:
            nc.sync.dma_start(
                out=AP(out_d, c * WLC * 256, [[32 * C, 128], [1, WLC * 256]]),
                in_=AP(outc.tensor, 0, [[WLC * 256, 128], [1, WLC * 256]]),
            )


def _get_program(n_repeat=1, with_bias=True):
    key = ("nc", n_repeat, with_bias)
    if key not in _cache:
        _cache[key] = _build_program(n_repeat, with_bias)
    return _cache[key]


def _make_in_maps(inputs):
    x, W1, b1, W2, b2 = (inputs[k] for k in ("x", "W1", "b1", "W2", "b2"))
    bf = ml_dtypes.bfloat16
    Wc = (np.asarray(W1, np.float32) @ np.asarray(W2, np.float32))
    bc = (np.asarray(b1, np.float32) @ np.asarray(W2, np.float32)
          + np.asarray(b2, np.float32))
    wc_h = np.ascontiguousarray(Wc.astype(bf).reshape(2, 128, M144))
    bc_h = np.ascontiguousarray(bc.astype(bf).reshape(1, M144))
    xb = np.asarray(x).astype(bf)
    xp = np.zeros((NCORES, PW, PW, C), dtype=bf)
    xp[:, 1:65, 1:65, :] = xb
    # xT[i, half, c, (hl, wl, hb, wb)] = x[i, hb*4+hl, wb*8+wl, half*128+c]
    xt = (xb.reshape(NCORES, 16, 4, 8, 8, 2, 128)
          .transpose(0, 5, 6, 2, 4, 1, 3)      # i, half, c, hl, wl, hb, wb
          .reshape(NCORES, 2, 128, XTF))
    return [
        {
            "xpad": np.ascontiguousarray(xp[i]),
            "xt": np.ascontiguousarray(xt[i]),
            "wc": wc_h,
            "bc": bc_h,
        }
        for i in range(NCORES)
    ]


def _unpermute(raw):
    # [128, 32*256] p-major -> [64, 64, 256]: p = hb*8+wb, free = (hl, co, wl)
    return (np.asarray(raw).reshape(16, 8, 4, 256, 8)
            .transpose(0, 2, 1, 4, 3)
            .reshape(64, 64, 256))


def kernel(x, W1, b1, W2, b2, trace=False):
    from concourse.bass_utils import run_bass_kernel_spmd

    bc = np.asarray(b1, np.float32) @ np.asarray(W2, np.float32) + np.asarray(
        b2, np.float32)
    nc = _get_program(with_bias=bool(np.any(bc != 0.0)))
    in_maps = _make_in_maps(dict(x=x, W1=W1, b1=b1, W2=W2, b2=b2))
    res = run_bass_kernel_spmd(nc, in_maps, core_ids=list(range(NCORES)),
                               trace=trace)
    out = np.stack([_unpermute(res.results[i]["out"]) for i in range(NCORES)],
                   axis=0).astype(np.float32)
    if trace:
        return out, res
    return out


# revision 75
# speedup vs baseline: 1.0408x; 1.0068x over previous
# Trainium2 Bass kernel for nn_Invo2D, v7: host-transposed xT (no PE
# transposes), row-merged product pieces, DVE/Pool product split, PE/DVE fold
# split, DMA spread across engine queues, fine-grained last-chunk tail.
#
#   t2 = x @ Wc + bc     (Wc = W1@W2, bc = b1@W2+b2 host-folded, bf16)
#   out[pix, co] = sum_j t2[pix, 9*(co//16)+j] * x_tap(f)[pix, ch(f)], f = 9*co+j
#
# Per core (1 image): partition p = hb*8 + wb (16 h-blocks x 8 w-blocks);
# each partition owns a 4x8 pixel block with a 1-pixel 2D halo: free dim =
# (6 h-slots x 10 w-slots x 256 ch).  x zero-padded to [66,66,256] bf16 on
# host.  X flat addr for product f of chunk c at wl: c*2560 + row*1792 +
# wl*256 + f, where row = (f//768); pieces only cut at tap-row boundaries
# (f=768,1536) and group boundaries -> 22 pieces/chunk.
# t2 matmuls use a host-prepared transposed copy xT [2,128,4096] (ch-major),
# so PE does no transposes.  Fold: PE identity-lhsT PSUM-accumulate for co
# [0, TREE_CO0), DVE/Pool bf16 add-tree for the rest; Act evacuates PSUM.
import numpy as np
import ml_dtypes

H, W, C = 64, 64, 256
M144 = 144
NCORES = 8
HS, WS = 6, 10          # h-slots, w-slots per partition (4+2, 8+2)
XF = HS * WS * C        # 15360 bf16 per partition
WLC = 8                 # wl positions per chunk = one hl row
NCHUNK = 4
M9F = WLC * 2304        # 18432
W9F = WLC * M144        # 1152
PW = 66                 # padded row length
XTF = 4096              # pixels per xT half

# fold split: PE folds co [0, TREE_CO0[c]) in 64-blocks, DVE tree the rest.
# Last chunk: PE tail in 16-co blocks with per-block stores and an all-DVE
# tree for the final co range so the tail chain has no cross-engine sems.
TREE_CO0 = (208, 208, 192, 224)
# product piece routing: relative rates (ns per free elem) + per-instr cost
DVE_NS = 0.52
POOL_NS = 0.833
DVE_INSTR_NS = 60.0
ROUTE_TD0_BIAS = 225.0   # empirically tuned DVE seed bias (sim sweep)

_cache = {}


# groups 5 and 10 straddle tap-row boundaries; their rect pieces
# (g, gc0, ngc, j0, nj, row):
_PARTIALS = {
    5: [(5, 0, 5, 0, 9, 0), (5, 5, 1, 0, 3, 0),
        (5, 5, 1, 3, 6, 1), (5, 6, 10, 0, 9, 1)],
    10: [(10, 0, 10, 0, 9, 1), (10, 10, 1, 0, 6, 1),
         (10, 10, 1, 6, 3, 2), (10, 11, 5, 0, 9, 2)],
}
_ROW_OF_FULL = {g: (0 if g < 5 else (1 if g < 10 else 2)) for g in range(16)
                if g not in (5, 10)}

def _build_pieces():
    """Pieces (g, gc0, ngc, j0, nj, row) in f order: full groups stay whole;
    groups 5 and 10 split into rects at tap-row boundaries."""
    pieces = []
    for g in range(16):
        if g in _PARTIALS:
            pieces.extend(_PARTIALS[g])
        else:
            pieces.append((g, 0, 16, 0, 9, _ROW_OF_FULL[g]))
    return pieces


def _route_pieces(pieces, td0=0.0, tp0=0.0):
    """Greedy DVE/Pool routing: assign each piece (f-ascending) to the engine
    with the lower projected finish time.  td0/tp0 seed the per-chunk fold
    burden each engine already carries (DVE: T1+T2 tree, Pool: T3+final).
    DVE pays a per-instruction SBUF-access cost; Pool does not (v1 model)."""
    td, tp = td0, tp0
    routed = []
    for pc in pieces:
        fe = WLC * pc[2] * pc[4]
        if td + fe * DVE_NS + DVE_INSTR_NS <= tp + fe * POOL_NS:
            td += fe * DVE_NS + DVE_INSTR_NS
            routed.append(("dve",) + pc)
        else:
            tp += fe * POOL_NS
            routed.append(("pool",) + pc)
    return routed


def _merge_dve_runs(routed, protect_g=-1):
    """Merge consecutive full-group DVE pieces with consecutive g, same tap
    row, AND the same fold block (g//4) — such merges cannot delay any PE
    fold block.  Merged entries get ng>1 (emitted with a 5-dim W9 AP)."""
    out = []
    for (eng, g, gc0, ngc, j0, nj, row) in routed:
        full = gc0 == 0 and ngc == 16 and j0 == 0 and nj == 9
        if (out and eng == "dve" and full and g > protect_g):
            peng, pg, png, pgc0, pngc, pj0, pnj, prow = out[-1]
            if (peng == "dve" and pgc0 == 0 and pngc == 16 and pnj == 9
                    and pg + png == g and prow == row
                    and pg // 4 == g // 4 and pg > protect_g):
                out[-1] = (peng, pg, png + 1, 0, 16, 0, 9, row)
                continue
        out.append((eng, g, 1, gc0, ngc, j0, nj, row))
    return out


def _build_program(n_repeat=1, with_bias=True):
    import concourse.bass as bass
    import concourse.tile as tile
    from concourse import bacc, mybir
    from concourse.masks import make_identity

    f32 = mybir.dt.float32
    bf16 = mybir.dt.bfloat16
    AP = bass.AP

    nc = bacc.Bacc(None, target_bir_lowering=False)
    x_d = nc.dram_tensor("xpad", [PW, PW, C], bf16, kind="ExternalInput")
    xt_d = nc.dram_tensor("xt", [2, 128, XTF], bf16, kind="ExternalInput")
    wc_d = nc.dram_tensor("wc", [2, 128, M144], bf16, kind="ExternalInput")
    bc_d = nc.dram_tensor("bc", [1, M144], bf16, kind="ExternalInput")
    out_d = nc.dram_tensor("out", [128, 32 * C], bf16, kind="ExternalOutput")

    pieces = _build_pieces()
    ROUTED = []
    for c in range(NCHUNK):
        nco_tree = 256 - TREE_CO0[c]
        td0 = (32 + 16) * nco_tree * DVE_NS + ROUTE_TD0_BIAS
        tp0 = (8 + 8) * nco_tree * POOL_NS        # T3 + final on Pool
        routed = _route_pieces(pieces, td0, tp0)
        # NOTE: _merge_dve_runs' 5-dim W9 APs pass CoreSim but neuronxcc's
        # codegen only supports TENSOR3D (3 free dims) for tensor ops --
        # keep pieces unmerged (measured equal anyway).
        ROUTED.append([(e, g, 1, gc0, ngc, j0, nj, row)
                       for (e, g, gc0, ngc, j0, nj, row) in routed])

    with tile.TileContext(nc) as tc:
        with (
            tc.tile_pool(name="singles", bufs=1) as singles,
            tc.tile_pool(name="xbufs", bufs=1) as xbufs,
            tc.tile_pool(name="w9p", bufs=1) as w9p,
            tc.tile_pool(name="m9p", bufs=3) as m9p,
            tc.tile_pool(name="treep", bufs=2) as treep,
            tc.tile_pool(name="outp", bufs=2) as outp,
            tc.tile_pool(name="ps2p", bufs=3, space="PSUM") as ps2p,
            tc.tile_pool(name="foldp", bufs=3, space="PSUM") as foldp,
        ):
            ident = singles.tile([128, 128], bf16)
            wc01 = singles.tile([128, 2 * M144], bf16)
            bcsb = singles.tile([1, M144], bf16)
            ones1 = singles.tile([1, 128], bf16)
            XT0 = singles.tile([128, XTF], bf16)
            XT1 = singles.tile([128, XTF], bf16)
            W9c = [w9p.tile([128, W9F], bf16, name=f"w9_{c}", tag=f"w9_{c}")
                   for c in range(NCHUNK)]

            for _rep in range(n_repeat):
                _image_body(nc, bass, mybir, make_identity,
                            xbufs, m9p, treep, outp, ps2p, foldp,
                            ident, wc01, bcsb, ones1, XT0, XT1, W9c,
                            x_d, xt_d, wc_d, bc_d, out_d, ROUTED, with_bias)
    nc.compile()
    return nc


def _image_body(nc, bass, mybir, make_identity,
                xbufs, m9p, treep, outp, ps2p, foldp,
                ident, wc01, bcsb, ones1, XT0, XT1, W9c,
                x_d, xt_d, wc_d, bc_d, out_d, ROUTED, with_bias):
    f32 = mybir.dt.float32
    bf16 = mybir.dt.bfloat16
    AP = bass.AP

    X = xbufs.tile([128, XF], bf16, tag="X")
    XTs = (XT0, XT1)

    def xt_load(eng, h, hl0, nhl, half=None):
        off, n = hl0 * 1024, nhl * 1024
        if half is not None:
            off, n = off + half * 512, 512
        eng.dma_start(
            out=AP(XTs[h].tensor, off, [[XTF, 128], [1, n]]),
            in_=AP(xt_d, h * 128 * XTF + off, [[XTF, 128], [1, n]]),
        )

    def x_slice(eng, s):
        eng.dma_start(
            out=AP(X.tensor, s * WS * C, [[XF, 128], [1, WS * C]]),
            in_=AP(x_d, s * PW * C,
                   [[4 * PW * C, 16], [8 * C, 8], [1, WS * C]]),
        )

    # -- prologue DMAs spread across queues (SP, Act, Pool) --
    # chunk-c row-r product pieces read only hslot c+r, so for chunk 0 the
    # slice deadlines relax: s0 ~4us, s1 ~5.5us, s2 ~7us.  Act's queue is
    # [wc0, wc1, <auto ATL>, evacs] so the act-table load overlaps t2.
    xt_load(nc.sync, 0, 0, 1)            # SP head: xT half0 row0
    xt_load(nc.scalar, 1, 0, 1)          # Act head: xT half1 row0 (parallel)
    # both wc halves in one DMA on Pool (delivery ~2.4us < first matmul)
    nc.gpsimd.dma_start(
        out=wc01[:],
        in_=AP(wc_d, 0, [[M144, 128], [128 * M144, 2], [1, M144]]),
    )
    if with_bias:
        nc.scalar.dma_start(out=bcsb[:], in_=bc_d[:])
        nc.vector.memset(ones1[:], 1.0)
    make_identity(nc, ident[:])          # Pool, ~0.2us
    x_slice(nc.sync, 0)
    x_slice(nc.gpsimd, 1)                # Pool DMA stream, parallel to SP
    xt_load(nc.sync, 0, 1, 1)
    xt_load(nc.sync, 1, 1, 1)
    x_slice(nc.gpsimd, 3)
    xt_load(nc.sync, 0, 2, 1)
    xt_load(nc.sync, 1, 2, 1)
    x_slice(nc.sync, 4)
    xt_load(nc.sync, 0, 3, 1)
    xt_load(nc.sync, 1, 3, 1)
    x_slice(nc.sync, 5)

    def t2_grp(r, split_evac=False):
        # 4 pair-tiles; 2 tiles share one PSUM tile, evacuated 288 wide.
        # grp0: evac pairs 0,1 on Act and 2,3 on DVE in parallel (W9c[0]
        # gates all products; DVE idles until then anyway).
        for q in range(4):
            ps2 = ps2p.tile([128, 288], f32, padded_shape=[128, 512])
            for i in range(2):
                t = r * 8 + 2 * q + i
                dst = ps2[:, i * M144:(i + 1) * M144]
                nc.tensor.matmul(dst, lhsT=AP(XT0.tensor, t * 128,
                                              [[XTF, 128], [1, 128]]),
                                 rhs=wc01[:, 0:M144], start=True, stop=False)
                nc.tensor.matmul(dst, lhsT=AP(XT1.tensor, t * 128,
                                              [[XTF, 128], [1, 128]]),
                                 rhs=wc01[:, M144:2 * M144], start=False,
                                 stop=not with_bias)
                if with_bias:
                    nc.tensor.matmul(dst, lhsT=ones1[:], rhs=bcsb[:],
                                     start=False, stop=True)
            w9dst = AP(W9c[r].tensor, 2 * q * M144, [[W9F, 128], [1, 288]])
            if split_evac and q % 2 == 1:
                nc.vector.tensor_copy(out=w9dst, in_=ps2[:])
            else:
                nc.scalar.copy(out=w9dst, in_=ps2[:])

    t2_grp(0, split_evac=True)
    # Act then loads x slice 2 (needed by chunk-0 row-2 products ~8us)
    x_slice(nc.scalar, 2)
    t2_grp(1)

    for c in range(NCHUNK):
        M9 = m9p.tile([128, M9F], bf16, tag="m9")
        # products, f-ascending, routed DVE/Pool.  Chunk 0's first pieces are
        # split into wl halves: the wl 0-3 half only reads W9 pairs 0-1, so
        # products start before the last grp-0 evacuations land.
        for (eng_name, g, ng, gc0, ngc, j0, nj, row) in ROUTED[c]:
            base = c * 2560 + row * 1792
            f0 = 144 * g + 9 * gc0 + j0
            eng = nc.vector if eng_name == "dve" else nc.gpsimd
            if ng > 1:
                # merged full-group DVE run: f-contiguous X/M9, 5-dim W9
                eng.tensor_mul(
                    AP(M9.tensor, f0, [[M9F, 128], [2304, WLC], [1, 144 * ng]]),
                    AP(X.tensor, base + f0,
                       [[XF, 128], [256, WLC], [1, 144 * ng]]),
                    AP(W9c[c].tensor, 9 * g,
                       [[W9F, 128], [M144, WLC], [9, ng], [0, 16], [1, 9]]),
                )
                continue
            halves = ((0, 4), (4, 4)) if (c == 0 and g <= 2) else ((0, WLC),)
            for wl0, nwl in halves:
                eng.tensor_mul(
                    AP(M9.tensor, wl0 * 2304 + f0,
                       [[M9F, 128], [2304, nwl], [9, ngc], [1, nj]]),
                    AP(X.tensor, base + wl0 * 256 + f0,
                       [[XF, 128], [256, nwl], [9, ngc], [1, nj]]),
                    AP(W9c[c].tensor, wl0 * M144 + 9 * g + j0,
                       [[W9F, 128], [M144, nwl], [0, ngc], [1, nj]]),
                )

        outc = outp.tile([128, WLC * 256], bf16, tag="outc")
        tree_co0 = TREE_CO0[c]

        # PE fold blocks for co [0, tree_co0); last chunk uses fine blocks so
        # PE tracks the product stream and the tail chain is short
        pe_blocks = []
        if c == NCHUNK - 1:
            pe_blocks += [(b, 32) for b in range(0, tree_co0, 32)]
        else:
            for co0 in range(0, tree_co0, 64):
                pe_blocks.append((co0, min(64, tree_co0 - co0)))
        for co0, nco in pe_blocks:
            ps = foldp.tile([128, 512], f32, padded_shape=[128, 512])
            for j in range(9):
                nc.tensor.matmul(
                    ps[:, :WLC * nco],
                    lhsT=ident[:],
                    rhs=AP(M9.tensor, 9 * co0 + j,
                           [[M9F, 128], [2304, WLC], [9, nco]]),
                    start=(j == 0),
                    stop=(j == 8),
                )
            # outc is co-major: outc[p, co*8 + wl]
            nc.scalar.copy(
                out=AP(outc.tensor, co0 * WLC,
                       [[WLC * 256, 128], [1, WLC], [WLC, nco]]),
                in_=AP(ps.tensor, 0, [[512, 128], [1, nco * WLC]]),
            )

        # interleave t2 for the next-next group while folds run
        if c + 2 < NCHUNK:
            t2_grp(c + 2)

        # DVE/Pool add-tree for co [tree_co0, 256).  Normal chunks: T1/T2 on
        # DVE, T3/final on Pool.  Last chunk: TWO independent half-trees run
        # in parallel (DVE and Pool), halving the serial tail fold.
        last = c == NCHUNK - 1
        def add_tree(co0, nco, tag, e12, e34):
            T1 = treep.tile([128, 32 * nco], bf16, tag=f"tree1{tag}")
            T2 = treep.tile([128, 16 * nco], bf16, tag=f"tree2{tag}")
            T3 = treep.tile([128, 8 * nco], bf16, tag=f"tree3{tag}")
            # T1[p] = M9[j=p] + M9[j=p+4], p=0..3 (last dims stride 1 -> 2x)
            e12.tensor_add(
                AP(T1.tensor, 0, [[32 * nco, 128], [4 * nco, WLC], [4, nco], [1, 4]]),
                AP(M9.tensor, 9 * co0, [[M9F, 128], [2304, WLC], [9, nco], [1, 4]]),
                AP(M9.tensor, 9 * co0 + 4, [[M9F, 128], [2304, WLC], [9, nco], [1, 4]]),
            )
            e12.tensor_add(
                AP(T2.tensor, 0, [[16 * nco, 128], [2 * nco, WLC], [2, nco], [1, 2]]),
                AP(T1.tensor, 0, [[32 * nco, 128], [4 * nco, WLC], [4, nco], [1, 2]]),
                AP(T1.tensor, 2, [[32 * nco, 128], [4 * nco, WLC], [4, nco], [1, 2]]),
            )
            e34.tensor_add(
                AP(T3.tensor, 0, [[8 * nco, 128], [nco, WLC], [1, nco]]),
                AP(T2.tensor, 0, [[16 * nco, 128], [2 * nco, WLC], [2, nco]]),
                AP(T2.tensor, 1, [[16 * nco, 128], [2 * nco, WLC], [2, nco]]),
            )
            e34.tensor_add(
                AP(outc.tensor, co0 * WLC,
                   [[WLC * 256, 128], [1, WLC], [WLC, nco]]),
                AP(T3.tensor, 0, [[8 * nco, 128], [nco, WLC], [1, nco]]),
                AP(M9.tensor, 9 * co0 + 8, [[M9F, 128], [2304, WLC], [9, nco]]),
            )
        nco = 256 - tree_co0
        if nco:
            if last and nco >= 32:
                half = nco // 2
                add_tree(tree_co0, half, "a", nc.vector, nc.vector)
                add_tree(tree_co0 + half, nco - half, "b", nc.gpsimd,
                         nc.gpsimd)
            else:
                add_tree(tree_co0, nco, "", nc.vector, nc.gpsimd)

        # store chunk (SP); last chunk: co [0,192) as one store, then the
        # 16-co tail blocks individually right after each evac (last two on
        # Act so they don't queue behind SP's big store)
        if c == NCHUNK - 1:
            for eng, (s0, sn) in [(nc.sync, (0, 128)), (nc.sync, (128, 64)),
                                  (nc.scalar, (192, 32)), (nc.sync, (224, 32))]:
                eng.dma_start(
                    out=AP(out_d, c * WLC * 256 + s0 * WLC,
                           [[32 * C, 128], [1, sn * WLC]]),
                    in_=AP(outc.tensor, s0 * WLC,
                           [[WLC * 256, 128], [1, sn * WLC]]),
                )
        else:
            nc.sync.dma_start(
                out=AP(out_d, c * WLC * 256, [[32 * C, 128], [1, WLC * 256]]),
                in_=AP(outc.tensor, 0, [[WLC * 256, 128], [1, WLC * 256]]),
            )


def _get_program(n_repeat=1, with_bias=True):
    key = ("nc", n_repeat, with_bias)
    if key not in _cache:
        _cache[key] = _build_program(n_repeat, with_bias)
    return _cache[key]


def _make_in_maps(inputs):
    x, W1, b1, W2, b2 = (inputs[k] for k in ("x", "W1", "b1", "W2", "b2"))
    bf = ml_dtypes.bfloat16
    Wc = (np.asarray(W1, np.float32) @ np.asarray(W2, np.float32))
    bc = (np.asarray(b1, np.float32) @ np.asarray(W2, np.float32)
          + np.asarray(b2, np.float32))
    wc_h = np.ascontiguousarray(Wc.astype(bf).reshape(2, 128, M144))
    bc_h = np.ascontiguousarray(bc.astype(bf).reshape(1, M144))
    xb = np.asarray(x).astype(bf)
    xp = np.zeros((NCORES, PW, PW, C), dtype=bf)
    xp[:, 1:65, 1:65, :] = xb
    # xT[i, half, c, (hl, wl, hb, wb)] = x[i, hb*4+hl, wb*8+wl, half*128+c]
    xt = (xb.reshape(NCORES, 16, 4, 8, 8, 2, 128)
          .transpose(0, 5, 6, 2, 4, 1, 3)      # i, half, c, hl, wl, hb, wb
          .reshape(NCORES, 2, 128, XTF))
    return [
        {
            "xpad": np.ascontiguousarray(xp[i]),
            "xt": np.ascontiguousarray(xt[i]),
            "wc": wc_h,
            "bc": bc_h,
        }
        for i in range(NCORES)
    ]


def _unpermute(raw):
    # [128, 32*256] p-major -> [64, 64, 256]: p = hb*8+wb, free = (hl, co, wl)
    return (np.asarray(raw).reshape(16, 8, 4, 256, 8)
            .transpose(0, 2, 1, 4, 3)
            .reshape(64, 64, 256))


def kernel(x, W1, b1, W2, b2, trace=False):
    from concourse.bass_utils import run_bass_kernel_spmd

    bc = np.asarray(b1, np.float32) @ np.asarray(W2, np.float32) + np.asarray(
        b2, np.float32)
    nc = _get_program(with_bias=bool(np.any(bc != 0.0)))
    in_maps = _make_in_maps(dict(x=x, W1=W1, b1=b1, W2=W2, b2=b2))
    res = run_bass_kernel_spmd(nc, in_maps, core_ids=list(range(NCORES)),
                               trace=trace)
    out = np.stack([_unpermute(res.results[i]["out"]) for i in range(NCORES)],
                   axis=0).astype(np.float32)
    if trace:
        return out, res
    return out
